# revision 9
# baseline (speedup 1.0000x reference)
"""ECG spiking encoder (conv-tokenizer + 2x {linear, parametric-LIF} + time-mean)
as a Bass kernel on 8 TRN2 NeuronCores, pure data parallel over batch.

Math (per core, batch shard of 64):
  patches   = im2col(x)                      # stride==kernel -> pure relayout
  h1        = patches @ Wc.T + bc            # conv fused with fc1 (host weight fold)
  u1        = sig1*h1 + sig1*bc              # folded into GEMM weights + epilogue bias
  LIF1      : v <- v + (h1 - v)*sig1 ; s = H(v-1) ; v <- v - s
  h2/u2     = fc2(s1) ...
  LIF2      ; out = mean_t(s2)

Device mapping:
  GEMM1: bf16 hi/lo 3-pass (exact to ~2^-16) over k=640 (5 chunks of 128, zero pad)
  GEMM2: float32r
  LIF   : one fused custom DVE op per step, both layers merged in one [128,128] tile
          state v'_t = (v'_{t-1} - (v'_{t-1} > 1)) * a + u_t   (v'-form, u pre-scaled)
  spikes: bulk  s = (v' > 1)  extraction; mean via tensor_reduce over t.
"""
import numpy as np
import ml_dtypes
from contextlib import ExitStack

import concourse.bass as bass
import concourse.tile as tile
from concourse import bacc, mybir
from concourse.bass_utils import run_bass_kernel_spmd

F32 = mybir.dt.float32
F32R = mybir.dt.float32r
BF16 = mybir.dt.bfloat16

# ---- problem constants (hardcoded per contract) ----
B, C, L = 512, 12, 5000
E, H1, H2, P = 128, 128, 128, 50
T = 100
STRIDE = 50
V_TH = 1.0
NCORES = 8
BS = B // NCORES          # 64 batch per core
K = C * P                 # 600 contraction
KPAD = 640                # 5 chunks of 128
NCH = KPAD // 128         # 5
NT = 13                   # row tiles: 12x512 + 1x256
ROWS = T * BS             # 6400
LAG = 8                   # layer-2 lags layer-1 by 8 steps (one block)
NBLK = T // 8             # 12.5 -> handled as 13 blocks (last half)
MSTEPS = T + LAG          # 108 merged scan steps


def _register_lif_op():
    """Fused LIF step as a custom DVE op, via the documented extension point
    (concourse dve_ops registry). Idempotent across calls."""
    import concourse.dve_ops as dom
    from concourse.dve_spec import Spec, Src0, Src1, C0, C1, lower, _has_src1
    from concourse.dve_uop import DveOpSpec

    name = "LIF_EMA_RESET_STEP"
    for op in dom.OPS:
        if op.name == name:
            return op

    body = (Src0 - (Src0 > C1)) * C0 + Src1

    def ref(in0, in1, s0, s1, imm2):
        return (((in0 - (in0 > s1)) * s0) + in1).astype(np.float32)

    spec = Spec(body=body, reference=ref)
    row = dom._CUSTOM_DVE_ROW_BASE + len(dom.OPS)
    assert row < 0x20
    shas = {}
    for ver in ("v3", "v4"):
        uops = lower(spec, ver=ver)
        shas[ver] = DveOpSpec(name=name, opcode=row, uops=uops,
                              rd1_en=_has_src1(spec)).sha(ver)
    op = dom.DveOp(name, spec, subdim=False, uops_sha=shas)
    dom.OPS.append(op)
    dom._SUB_OPCODE_FOR_NAME[name] = row
    dom.CUSTOM_DVE_SPECS[name] = spec
    return op


def _build_program(a1: float, a2: float):
    lif_op = _register_lif_op()
    nc = bacc.Bacc("TRN2", target_bir_lowering=False, debug=False,
                   num_devices=NCORES)

    xhl_d = nc.dram_tensor("xhl", [2 * NCH, 128, ROWS], BF16, kind="ExternalInput").ap()
    w1_d = nc.dram_tensor("w1", [2 * NCH, 128, H1], BF16, kind="ExternalInput").ap()
    b1_d = nc.dram_tensor("b1", [128, 1], F32, kind="ExternalInput").ap()
    w2_d = nc.dram_tensor("w2", [128, H2], F32R, kind="ExternalInput").ap()
    b2_d = nc.dram_tensor("b2", [128, 1], F32, kind="ExternalInput").ap()
    out_d = nc.dram_tensor("out", [128, BS], F32, kind="ExternalOutput").ap()

    # GEMM1 3-pass MM schedule: hi*Wh (5), lo*Wh (5), hi*Wl (5)
    W_IDX = [0, 1, 2, 3, 4] + [0, 1, 2, 3, 4] + [5, 6, 7, 8, 9]
    X_IDX = [0, 1, 2, 3, 4] + [5, 6, 7, 8, 9] + [0, 1, 2, 3, 4]

    # staircase column groups: small first so the scan pipeline starts early
    GROUPS = [(0, 512), (512, 1024), (1536, 1536), (3072, 1536), (4608, 1536), (6144, 256)]
    HALF = MSTEPS * 64          # vball column offset of the layer-2 half

    with tile.TileContext(nc) as tc, ExitStack() as ctx:
        wpool = ctx.enter_context(tc.tile_pool(name="wpool", bufs=1))
        xpool = ctx.enter_context(tc.tile_pool(name="xpool", bufs=3))
        upool = ctx.enter_context(tc.tile_pool(name="upool", bufs=6))
        spool = ctx.enter_context(tc.tile_pool(name="spool", bufs=3))
        vpool = ctx.enter_context(tc.tile_pool(name="vpool", bufs=1))
        ps1pool = ctx.enter_context(tc.tile_pool(name="ps1", bufs=4, space="PSUM"))
        ps2pool = ctx.enter_context(tc.tile_pool(name="ps2", bufs=2, space="PSUM"))
        mpool = ctx.enter_context(tc.tile_pool(name="mpool", bufs=1))

        # weights
        wt = wpool.tile([128, 10 * H1], BF16)
        for i in range(10):
            nc.scalar.dma_start(wt[:, bass.ts(i, H1)], w1_d[i])
        w2t = wpool.tile([128, H2], F32R)
        nc.scalar.dma_start(w2t[:], w2_d[:])
        b1t = wpool.tile([128, 1], F32)
        nc.scalar.dma_start(b1t[:], b1_d[:])
        b2t = wpool.tile([128, 1], F32)
        nc.scalar.dma_start(b2t[:], b2_d[:])

        # merged v' trajectory, split halves: L1 at cols [0, HALF), L2 at [HALF, 2*HALF)
        vball = vpool.tile([128, 2 * HALF], F32)
        vb2 = vball[:].rearrange("p (h q) -> p h q", h=2)
        zinit = wpool.tile([128, 128], F32)
        nc.vector.memset(zinit[:], 0.0)

        # u blocks: [128, 1024] = u1 (8 steps x 64) | u2 (8 steps x 64)
        ublks = [None] * 14

        def ublk_for(k):
            if ublks[k] is None:
                t_ = upool.tile([128, 1024], F32, tag="ublk", name=f"ublk{k}")
                ublks[k] = t_
            return ublks[k]

        m_done = 0
        merged = float(a1) == float(a2)

        def emit_scan_through(m_end):
            nonlocal m_done
            while m_done < m_end:
                m = m_done
                ub = ublks[m // 8]
                ub2 = ub[:].rearrange("p (h q) -> p h q", h=2)
                s = m % 8
                if merged:
                    src = (zinit[:].rearrange("p (h q) -> p h q", h=2) if m == 0
                           else vb2[:, :, (m - 1) * 64:m * 64])
                    nc.vector._custom_dve(
                        lif_op, out=vb2[:, :, m * 64:(m + 1) * 64], in0=src,
                        in1=ub2[:, :, s * 64:(s + 1) * 64], s0=a1, s1=V_TH)
                else:
                    for h, a_ in ((0, a1), (1, a2)):
                        src = (zinit[:, 0:64] if m == 0
                               else vball[:, h * HALF + (m - 1) * 64:h * HALF + m * 64])
                        nc.vector._custom_dve(
                            lif_op,
                            out=vball[:, h * HALF + m * 64:h * HALF + (m + 1) * 64],
                            in0=src,
                            in1=ub[:, h * 512 + s * 64:h * 512 + (s + 1) * 64],
                            s0=a_, s1=V_TH)
                m_done += 1

        tail1_done = False
        tail2_done = False

        for g, (gc0, gcn) in enumerate(GROUPS):
            tiles = []
            c = gc0
            while c < gc0 + gcn:
                w = min(512, gc0 + gcn - c)
                tiles.append((c // 512, c, w))
                c += w

            xg = xpool.tile([128, 10 * 1536], BF16, tag="xg", name=f"xg{g}")
            for i in range(10):
                eng = nc.sync if i % 2 == 0 else nc.scalar
                eng.dma_start(xg[:, i * 1536:i * 1536 + gcn],
                              xhl_d[i, :, gc0:gc0 + gcn])

            for (j, col0, ncols) in tiles:
                nsteps = ncols // 64
                goff = col0 - gc0
                ps = ps1pool.tile([128, ncols], F32, tag="ps1t", name=f"ps{j}")
                for i in range(15):
                    nc.tensor.matmul(
                        ps[:], wt[:, bass.ts(W_IDX[i], H1)],
                        xg[:, X_IDX[i] * 1536 + goff: X_IDX[i] * 1536 + goff + ncols],
                        start=(i == 0), stop=(i == 14))

                # epilogue 1 -> u1 half of block j
                ub = ublk_for(j)
                if j == 0:
                    nc.vector.memset(ub[:], 0.0)  # u2 of block 0 = 0
                nc.scalar.activation(
                    ub[:].rearrange("p (s c) -> p s c", c=64)[:, :nsteps],
                    ps[:].rearrange("p (s c) -> p s c", c=64),
                    mybir.ActivationFunctionType.Identity, bias=b1t[:, 0:1])
                if j == 12:
                    nc.vector.memset(ub[:, nsteps * 64:512], 0.0)

                emit_scan_through(min(8 * (j + 1), MSTEPS))

                # s1 extraction for block j (contiguous L1 half -> 2x mode)
                sb = spool.tile([128, 512], F32R, tag="s1b", name=f"s1b{j}")
                nc.vector.tensor_scalar(
                    sb[:, :ncols], vball[:, 8 * j * 64:8 * j * 64 + ncols],
                    V_TH, None, mybir.AluOpType.is_gt, mybir.AluOpType.bypass)

                # GEMM2 -> u2 for L2 steps 8j.. lands in u block j+2
                ps2 = ps2pool.tile([128, ncols], F32, tag="ps2t", name=f"ps2{j}")
                nc.tensor.matmul(ps2[:], w2t[:], sb[:, :ncols], start=True, stop=True)
                ub_next = ublk_for(j + 1)
                if j + 1 >= 13:
                    nc.vector.memset(ub_next[:, 0:512], 0.0)  # u1 of tail block = 0
                nc.scalar.activation(
                    ub_next[:].rearrange("p (s c) -> p s c", c=64)[:, 8:8 + nsteps],
                    ps2[:].rearrange("p (s c) -> p s c", c=64),
                    mybir.ActivationFunctionType.Identity, bias=b2t[:, 0:1])
                if j == 12 and nsteps < 8:
                    nc.vector.memset(ub_next[:, 512 + nsteps * 64:1024], 0.0)

                # early partial layer-2 tails once enough steps are done
                if m_done >= 66 and not tail1_done:
                    tail1_done = True
                    r0 = HALF + LAG * 64
                    nc.vector.tensor_scalar(
                        vball[:, r0:r0 + 3200], vball[:, r0:r0 + 3200],
                        V_TH, None, mybir.AluOpType.is_gt, mybir.AluOpType.bypass)
                    acc1 = mpool.tile([128, BS], F32, name="acc1")
                    nc.vector.tensor_reduce(
                        acc1[:],
                        vball[:, r0:r0 + 3200].rearrange("p (t b) -> p b t", b=64),
                        mybir.AxisListType.X, mybir.AluOpType.add)
                if m_done >= 104 and not tail2_done:
                    tail2_done = True
                    r0 = HALF + LAG * 64 + 3200
                    n2 = 38 * 64
                    nc.vector.tensor_scalar(
                        vball[:, r0:r0 + n2], vball[:, r0:r0 + n2],
                        V_TH, None, mybir.AluOpType.is_gt, mybir.AluOpType.bypass)
                    acc2 = mpool.tile([128, BS], F32, name="acc2")
                    nc.vector.tensor_reduce(
                        acc2[:],
                        vball[:, r0:r0 + n2].rearrange("p (t b) -> p b t", b=64),
                        mybir.AxisListType.X, mybir.AluOpType.add)
                    acc12 = mpool.tile([128, BS], F32, name="acc12")
                    nc.vector.scalar_tensor_tensor(
                        acc12[:], acc1[:], 1.0, acc2[:],
                        mybir.AluOpType.mult, mybir.AluOpType.add)

        emit_scan_through(MSTEPS)

        # last 12 t-steps of layer-2 spikes + mean
        r1 = HALF + LAG * 64 + 5632
        n3 = 12 * 64
        nc.vector.tensor_scalar(vball[:, r1:r1 + n3], vball[:, r1:r1 + n3],
                                V_TH, None, mybir.AluOpType.is_gt,
                                mybir.AluOpType.bypass)
        acc3 = mpool.tile([128, BS], F32, name="acc3")
        nc.vector.tensor_reduce(
            acc3[:], vball[:, r1:r1 + n3].rearrange("p (t b) -> p b t", b=64),
            mybir.AxisListType.X, mybir.AluOpType.add)
        acc = mpool.tile([128, BS], F32, name="accf")
        nc.vector.scalar_tensor_tensor(acc[:], acc12[:], 1.0, acc3[:],
                                       mybir.AluOpType.mult, mybir.AluOpType.add)
        nc.vector.tensor_scalar(acc[:], acc[:], float(np.float32(1.0 / T)), None,
                                mybir.AluOpType.mult, mybir.AluOpType.bypass)
        nc.sync.dma_start(out_d[:], acc[:])

    nc.compile()
    return nc


_PROG_CACHE = {}


def _get_program(a1, a2):
    key = (round(float(a1), 10), round(float(a2), 10))
    if key not in _PROG_CACHE:
        _PROG_CACHE[key] = _build_program(float(a1), float(a2))
    return _PROG_CACHE[key]


def kernel(x, conv_w, conv_b, fc1_w, fc1_b, fc2_w, fc2_b, w1, w2):
    x = np.asarray(x, np.float32)
    conv_w = np.asarray(conv_w, np.float32)
    conv_b = np.asarray(conv_b, np.float32)
    fc1_w = np.asarray(fc1_w, np.float32)
    fc1_b = np.asarray(fc1_b, np.float32)
    fc2_w = np.asarray(fc2_w, np.float32)
    fc2_b = np.asarray(fc2_b, np.float32)

    sig1 = 1.0 / (1.0 + np.exp(-np.float64(w1)))
    sig2 = 1.0 / (1.0 + np.exp(-np.float64(w2)))
    a1 = np.float32(1.0 - sig1)
    a2 = np.float32(1.0 - sig2)
    sig1 = np.float32(sig1)
    sig2 = np.float32(sig2)

    # ---- weight folding (host, fp64 for exactness headroom) ----
    # u1 = sig1*(fc1_w @ (conv_w.x + conv_b) + fc1_b)
    Wc = sig1.astype(np.float64) * (fc1_w.astype(np.float64) @ conv_w.reshape(E, K).astype(np.float64))
    bc = sig1.astype(np.float64) * (fc1_w.astype(np.float64) @ conv_b.astype(np.float64) + fc1_b.astype(np.float64))
    Wc = Wc.astype(np.float32)                      # [H1, K]
    bc = bc.astype(np.float32)                      # [H1]
    Wcp = np.zeros((H1, KPAD), np.float32)
    Wcp[:, :K] = Wc
    # lhsT chunks [k, H1], bf16 hi/lo
    WcT = Wcp.T.copy()                              # [KPAD, H1]
    Wh = WcT.astype(ml_dtypes.bfloat16)
    Wl = (WcT - Wh.astype(np.float32)).astype(ml_dtypes.bfloat16)
    w1_arr = np.concatenate([
        Wh.reshape(NCH, 128, H1), Wl.reshape(NCH, 128, H1)], axis=0)  # [10,128,H1]

    W2T = (sig2.astype(np.float64) * fc2_w.astype(np.float64)).T.astype(np.float32).copy()  # [H1, H2] lhsT
    b1_arr = bc.reshape(128, 1)
    b2_arr = (sig2 * fc2_b).astype(np.float32).reshape(128, 1)

    # ---- im2col + shard (pure relayout; stride == kernel width) ----
    # x [B, C, L] -> per-core [64, C, T, P] -> (c, p, t, b) -> [K, T*BS]
    in_maps = []
    for ci in range(NCORES):
        xs = x[ci * BS:(ci + 1) * BS].reshape(BS, C, T, P)
        xT = np.ascontiguousarray(xs.transpose(1, 3, 2, 0)).reshape(K, ROWS)
        xTp = np.zeros((KPAD, ROWS), np.float32)
        xTp[:K] = xT
        xh = xTp.astype(ml_dtypes.bfloat16)
        xl = (xTp - xh.astype(np.float32)).astype(ml_dtypes.bfloat16)
        xhl = np.concatenate([xh.reshape(NCH, 128, ROWS),
                              xl.reshape(NCH, 128, ROWS)], axis=0)
        in_maps.append({
            "xhl": xhl, "w1": w1_arr, "b1": b1_arr,
            "w2": W2T, "b2": b2_arr,
        })

    nc = _get_program(a1, a2)
    res = run_bass_kernel_spmd(nc, in_maps, list(range(NCORES)))

    out = np.empty((B, H2), np.float32)
    for ci in range(NCORES):
        out[ci * BS:(ci + 1) * BS] = res.results[ci]["out"].T
    return out


# revision 10
# speedup vs baseline: 1.1561x; 1.1561x over previous
"""ECG spiking encoder (conv-tokenizer + 2x {linear, parametric-LIF} + time-mean)
as a Bass kernel on 8 TRN2 NeuronCores, pure data parallel over batch.

Math (per core, batch shard of 64):
  patches   = im2col(x)                      # stride==kernel -> pure relayout
  h1        = patches @ Wc.T + bc            # conv fused with fc1 (host weight fold)
  u1        = sig1*h1 + sig1*bc              # folded into GEMM weights + epilogue bias
  LIF1      : v <- v + (h1 - v)*sig1 ; s = H(v-1) ; v <- v - s
  h2/u2     = fc2(s1) ...
  LIF2      ; out = mean_t(s2)

Device mapping:
  GEMM1: bf16 hi/lo 3-pass (exact to ~2^-16) over k=640 (5 chunks of 128, zero pad)
  GEMM2: float32r
  LIF   : one fused custom DVE op per step, both layers merged in one [128,128] tile
          state v'_t = (v'_{t-1} - (v'_{t-1} > 1)) * a + u_t   (v'-form, u pre-scaled)
  spikes: bulk  s = (v' > 1)  extraction; mean via tensor_reduce over t.
"""
import numpy as np
import ml_dtypes
from contextlib import ExitStack

import concourse.bass as bass
import concourse.tile as tile
from concourse import bacc, mybir
from concourse.bass_utils import run_bass_kernel_spmd

F32 = mybir.dt.float32
F32R = mybir.dt.float32r
BF16 = mybir.dt.bfloat16

# ---- problem constants (hardcoded per contract) ----
B, C, L = 512, 12, 5000
E, H1, H2, P = 128, 128, 128, 50
T = 100
STRIDE = 50
V_TH = 1.0
NCORES = 8
BS = B // NCORES          # 64 batch per core
K = C * P                 # 600 contraction
KPAD = 640                # 5 chunks of 128
NCH = KPAD // 128         # 5
NT = 13                   # row tiles: 12x512 + 1x256
ROWS = T * BS             # 6400
LAG = 16                  # layer-2 lag in steps (multiple of 8)
NBLK = T // 8             # 12.5 -> handled as 13 blocks (last half)
MSTEPS = T + LAG          # 108 merged scan steps


def _register_lif_op():
    """Fused LIF step as a custom DVE op, via the documented extension point
    (concourse dve_ops registry). Idempotent across calls."""
    import concourse.dve_ops as dom
    from concourse.dve_spec import Spec, Src0, Src1, C0, C1, lower, _has_src1
    from concourse.dve_uop import DveOpSpec

    name = "LIF_EMA_RESET_STEP"
    for op in dom.OPS:
        if op.name == name:
            return op

    body = (Src0 - (Src0 > C1)) * C0 + Src1

    def ref(in0, in1, s0, s1, imm2):
        return (((in0 - (in0 > s1)) * s0) + in1).astype(np.float32)

    spec = Spec(body=body, reference=ref)
    row = dom._CUSTOM_DVE_ROW_BASE + len(dom.OPS)
    assert row < 0x20
    shas = {}
    for ver in ("v3", "v4"):
        uops = lower(spec, ver=ver)
        shas[ver] = DveOpSpec(name=name, opcode=row, uops=uops,
                              rd1_en=_has_src1(spec)).sha(ver)
    op = dom.DveOp(name, spec, subdim=False, uops_sha=shas)
    dom.OPS.append(op)
    dom._SUB_OPCODE_FOR_NAME[name] = row
    dom.CUSTOM_DVE_SPECS[name] = spec
    return op


def _build_program(a1: float, a2: float):
    lif_op = _register_lif_op()
    nc = bacc.Bacc("TRN2", target_bir_lowering=False, debug=False,
                   num_devices=NCORES)

    xhl_d = nc.dram_tensor("xhl", [2 * NCH, 128, ROWS], BF16, kind="ExternalInput").ap()
    w1_d = nc.dram_tensor("w1", [2 * NCH, 128, H1], BF16, kind="ExternalInput").ap()
    b1_d = nc.dram_tensor("b1", [128, 1], F32, kind="ExternalInput").ap()
    w2_d = nc.dram_tensor("w2", [128, H2], F32R, kind="ExternalInput").ap()
    b2_d = nc.dram_tensor("b2", [128, 1], F32, kind="ExternalInput").ap()
    out_d = nc.dram_tensor("out", [128, BS], F32, kind="ExternalOutput").ap()

    # GEMM1 3-pass MM schedule: hi*Wh (5), lo*Wh (5), hi*Wl (5)
    W_IDX = [0, 1, 2, 3, 4] + [0, 1, 2, 3, 4] + [5, 6, 7, 8, 9]
    X_IDX = [0, 1, 2, 3, 4] + [5, 6, 7, 8, 9] + [0, 1, 2, 3, 4]

    # staircase column groups: small first so the scan pipeline starts early
    GROUPS = [(0, 512), (512, 1024), (1536, 1536), (3072, 1536), (4608, 1536), (6144, 256)]
    HALF = MSTEPS * 64          # vball column offset of the layer-2 half

    with tile.TileContext(nc) as tc, ExitStack() as ctx:
        wpool = ctx.enter_context(tc.tile_pool(name="wpool", bufs=1))
        xpool = ctx.enter_context(tc.tile_pool(name="xpool", bufs=3))
        upool = ctx.enter_context(tc.tile_pool(name="upool", bufs=6))
        spool = ctx.enter_context(tc.tile_pool(name="spool", bufs=3))
        vpool = ctx.enter_context(tc.tile_pool(name="vpool", bufs=1))
        ps1pool = ctx.enter_context(tc.tile_pool(name="ps1", bufs=4, space="PSUM"))
        ps2pool = ctx.enter_context(tc.tile_pool(name="ps2", bufs=2, space="PSUM"))
        mpool = ctx.enter_context(tc.tile_pool(name="mpool", bufs=1))

        # weights
        wt = wpool.tile([128, 10 * H1], BF16)
        for i in range(10):
            nc.scalar.dma_start(wt[:, bass.ts(i, H1)], w1_d[i])
        w2t = wpool.tile([128, H2], F32R)
        nc.scalar.dma_start(w2t[:], w2_d[:])
        b1t = wpool.tile([128, 1], F32)
        nc.scalar.dma_start(b1t[:], b1_d[:])
        b2t = wpool.tile([128, 1], F32)
        nc.scalar.dma_start(b2t[:], b2_d[:])

        # merged v' trajectory, split halves: L1 at cols [0, HALF), L2 at [HALF, 2*HALF)
        vball = vpool.tile([128, 2 * HALF], F32)
        vb2 = vball[:].rearrange("p (h q) -> p h q", h=2)
        zinit = wpool.tile([128, 128], F32)
        nc.vector.memset(zinit[:], 0.0)

        # u blocks: [128, 1024] = u1 (8 steps x 64) | u2 (8 steps x 64)
        ublks = [None] * (14 + LAG // 8)

        def ublk_for(k):
            if ublks[k] is None:
                t_ = upool.tile([128, 1024], F32, tag="ublk", name=f"ublk{k}")
                ublks[k] = t_
            return ublks[k]

        m_done = 0
        merged = float(a1) == float(a2)

        def emit_scan_through(m_end):
            nonlocal m_done
            while m_done < m_end:
                m = m_done
                ub = ublks[m // 8]
                ub2 = ub[:].rearrange("p (h q) -> p h q", h=2)
                s = m % 8
                if merged:
                    src = (zinit[:].rearrange("p (h q) -> p h q", h=2) if m == 0
                           else vb2[:, :, (m - 1) * 64:m * 64])
                    nc.vector._custom_dve(
                        lif_op, out=vb2[:, :, m * 64:(m + 1) * 64], in0=src,
                        in1=ub2[:, :, s * 64:(s + 1) * 64], s0=a1, s1=V_TH)
                else:
                    for h, a_ in ((0, a1), (1, a2)):
                        src = (zinit[:, 0:64] if m == 0
                               else vball[:, h * HALF + (m - 1) * 64:h * HALF + m * 64])
                        nc.vector._custom_dve(
                            lif_op,
                            out=vball[:, h * HALF + m * 64:h * HALF + (m + 1) * 64],
                            in0=src,
                            in1=ub[:, h * 512 + s * 64:h * 512 + (s + 1) * 64],
                            s0=a_, s1=V_TH)
                m_done += 1

        tail1_done = False
        tail2_done = False

        for g, (gc0, gcn) in enumerate(GROUPS):
            tiles = []
            c = gc0
            while c < gc0 + gcn:
                w = min(512, gc0 + gcn - c)
                tiles.append((c // 512, c, w))
                c += w

            xg = xpool.tile([128, 10 * 1536], BF16, tag="xg", name=f"xg{g}")
            for i in range(10):
                eng = nc.sync if i % 2 == 0 else nc.scalar
                eng.dma_start(xg[:, i * 1536:i * 1536 + gcn],
                              xhl_d[i, :, gc0:gc0 + gcn])

            for (j, col0, ncols) in tiles:
                nsteps = ncols // 64
                goff = col0 - gc0
                ps = ps1pool.tile([128, ncols], F32, tag="ps1t", name=f"ps{j}")
                for i in range(15):
                    nc.tensor.matmul(
                        ps[:], wt[:, bass.ts(W_IDX[i], H1)],
                        xg[:, X_IDX[i] * 1536 + goff: X_IDX[i] * 1536 + goff + ncols],
                        start=(i == 0), stop=(i == 14))

                # epilogue 1 -> u1 half of block j
                ub = ublk_for(j)
                if j < LAG // 8:
                    nc.vector.memset(ub[:], 0.0)  # u2 of first blocks = 0
                nc.scalar.activation(
                    ub[:].rearrange("p (s c) -> p s c", c=64)[:, :nsteps],
                    ps[:].rearrange("p (s c) -> p s c", c=64),
                    mybir.ActivationFunctionType.Identity, bias=b1t[:, 0:1])
                if j == 12:
                    nc.vector.memset(ub[:, nsteps * 64:512], 0.0)

                emit_scan_through(min(8 * (j + 1), MSTEPS))

                # s1 extraction for block j (contiguous L1 half -> 2x mode)
                sb = spool.tile([128, 512], F32R, tag="s1b", name=f"s1b{j}")
                nc.vector.tensor_scalar(
                    sb[:, :ncols], vball[:, 8 * j * 64:8 * j * 64 + ncols],
                    V_TH, None, mybir.AluOpType.is_gt, mybir.AluOpType.bypass)

                # GEMM2 -> u2 for L2 steps 8j.. lands in u block j+2
                ps2 = ps2pool.tile([128, ncols], F32, tag="ps2t", name=f"ps2{j}")
                nc.tensor.matmul(ps2[:], w2t[:], sb[:, :ncols], start=True, stop=True)
                ub_next = ublk_for(j + LAG // 8)
                if j + LAG // 8 >= 13:
                    nc.vector.memset(ub_next[:, 0:512], 0.0)  # u1 of tail blocks = 0
                nc.scalar.activation(
                    ub_next[:].rearrange("p (s c) -> p s c", c=64)[:, 8:8 + nsteps],
                    ps2[:].rearrange("p (s c) -> p s c", c=64),
                    mybir.ActivationFunctionType.Identity, bias=b2t[:, 0:1])
                if j == 12 and nsteps < 8:
                    nc.vector.memset(ub_next[:, 512 + nsteps * 64:1024], 0.0)

                # early partial layer-2 tails once enough steps are done
                if m_done >= 52 + LAG + 2 and not tail1_done:
                    tail1_done = True
                    r0 = HALF + LAG * 64
                    nc.vector.tensor_scalar(
                        vball[:, r0:r0 + 3200], vball[:, r0:r0 + 3200],
                        V_TH, None, mybir.AluOpType.is_gt, mybir.AluOpType.bypass)
                    acc1 = mpool.tile([128, BS], F32, name="acc1")
                    nc.vector.tensor_reduce(
                        acc1[:],
                        vball[:, r0:r0 + 3200].rearrange("p (t b) -> p b t", b=64),
                        mybir.AxisListType.X, mybir.AluOpType.add)
                if m_done >= 104 and not tail2_done:
                    tail2_done = True
                    r0 = HALF + LAG * 64 + 3200
                    n2 = 38 * 64
                    nc.vector.tensor_scalar(
                        vball[:, r0:r0 + n2], vball[:, r0:r0 + n2],
                        V_TH, None, mybir.AluOpType.is_gt, mybir.AluOpType.bypass)
                    acc2 = mpool.tile([128, BS], F32, name="acc2")
                    nc.vector.tensor_reduce(
                        acc2[:],
                        vball[:, r0:r0 + n2].rearrange("p (t b) -> p b t", b=64),
                        mybir.AxisListType.X, mybir.AluOpType.add)
                    acc12 = mpool.tile([128, BS], F32, name="acc12")
                    nc.vector.scalar_tensor_tensor(
                        acc12[:], acc1[:], 1.0, acc2[:],
                        mybir.AluOpType.mult, mybir.AluOpType.add)

        emit_scan_through(MSTEPS)

        # last 12 t-steps of layer-2 spikes + mean
        r1 = HALF + LAG * 64 + 5632
        n3 = 12 * 64
        nc.vector.tensor_scalar(vball[:, r1:r1 + n3], vball[:, r1:r1 + n3],
                                V_TH, None, mybir.AluOpType.is_gt,
                                mybir.AluOpType.bypass)
        acc3 = mpool.tile([128, BS], F32, name="acc3")
        nc.vector.tensor_reduce(
            acc3[:], vball[:, r1:r1 + n3].rearrange("p (t b) -> p b t", b=64),
            mybir.AxisListType.X, mybir.AluOpType.add)
        acc = mpool.tile([128, BS], F32, name="accf")
        nc.vector.scalar_tensor_tensor(acc[:], acc12[:], 1.0, acc3[:],
                                       mybir.AluOpType.mult, mybir.AluOpType.add)
        nc.vector.tensor_scalar(acc[:], acc[:], float(np.float32(1.0 / T)), None,
                                mybir.AluOpType.mult, mybir.AluOpType.bypass)
        nc.sync.dma_start(out_d[:], acc[:])

    nc.compile()
    return nc


_PROG_CACHE = {}


def _get_program(a1, a2):
    key = (round(float(a1), 10), round(float(a2), 10))
    if key not in _PROG_CACHE:
        _PROG_CACHE[key] = _build_program(float(a1), float(a2))
    return _PROG_CACHE[key]


def kernel(x, conv_w, conv_b, fc1_w, fc1_b, fc2_w, fc2_b, w1, w2):
    x = np.asarray(x, np.float32)
    conv_w = np.asarray(conv_w, np.float32)
    conv_b = np.asarray(conv_b, np.float32)
    fc1_w = np.asarray(fc1_w, np.float32)
    fc1_b = np.asarray(fc1_b, np.float32)
    fc2_w = np.asarray(fc2_w, np.float32)
    fc2_b = np.asarray(fc2_b, np.float32)

    sig1 = 1.0 / (1.0 + np.exp(-np.float64(w1)))
    sig2 = 1.0 / (1.0 + np.exp(-np.float64(w2)))
    a1 = np.float32(1.0 - sig1)
    a2 = np.float32(1.0 - sig2)
    sig1 = np.float32(sig1)
    sig2 = np.float32(sig2)

    # ---- weight folding (host, fp64 for exactness headroom) ----
    # u1 = sig1*(fc1_w @ (conv_w.x + conv_b) + fc1_b)
    Wc = sig1.astype(np.float64) * (fc1_w.astype(np.float64) @ conv_w.reshape(E, K).astype(np.float64))
    bc = sig1.astype(np.float64) * (fc1_w.astype(np.float64) @ conv_b.astype(np.float64) + fc1_b.astype(np.float64))
    Wc = Wc.astype(np.float32)                      # [H1, K]
    bc = bc.astype(np.float32)                      # [H1]
    Wcp = np.zeros((H1, KPAD), np.float32)
    Wcp[:, :K] = Wc
    # lhsT chunks [k, H1], bf16 hi/lo
    WcT = Wcp.T.copy()                              # [KPAD, H1]
    Wh = WcT.astype(ml_dtypes.bfloat16)
    Wl = (WcT - Wh.astype(np.float32)).astype(ml_dtypes.bfloat16)
    w1_arr = np.concatenate([
        Wh.reshape(NCH, 128, H1), Wl.reshape(NCH, 128, H1)], axis=0)  # [10,128,H1]

    W2T = (sig2.astype(np.float64) * fc2_w.astype(np.float64)).T.astype(np.float32).copy()  # [H1, H2] lhsT
    b1_arr = bc.reshape(128, 1)
    b2_arr = (sig2 * fc2_b).astype(np.float32).reshape(128, 1)

    # ---- im2col + shard (pure relayout; stride == kernel width) ----
    # x [B, C, L] -> per-core [64, C, T, P] -> (c, p, t, b) -> [K, T*BS]
    in_maps = []
    for ci in range(NCORES):
        xs = x[ci * BS:(ci + 1) * BS].reshape(BS, C, T, P)
        xT = np.ascontiguousarray(xs.transpose(1, 3, 2, 0)).reshape(K, ROWS)
        xTp = np.zeros((KPAD, ROWS), np.float32)
        xTp[:K] = xT
        xh = xTp.astype(ml_dtypes.bfloat16)
        xl = (xTp - xh.astype(np.float32)).astype(ml_dtypes.bfloat16)
        xhl = np.concatenate([xh.reshape(NCH, 128, ROWS),
                              xl.reshape(NCH, 128, ROWS)], axis=0)
        in_maps.append({
            "xhl": xhl, "w1": w1_arr, "b1": b1_arr,
            "w2": W2T, "b2": b2_arr,
        })

    nc = _get_program(a1, a2)
    res = run_bass_kernel_spmd(nc, in_maps, list(range(NCORES)))

    out = np.empty((B, H2), np.float32)
    for ci in range(NCORES):
        out[ci * BS:(ci + 1) * BS] = res.results[ci]["out"].T
    return out
